# revision 3
# baseline (speedup 1.0000x reference)
"""Trainium2 Bass kernel for DescriptorMatcher (mutual nearest neighbor matching).

Problem: given desc0 [B,N,D], desc1 [B,M,D] (B=4, N=M=8192, D=128, fp32):
    sim     = desc0 @ desc1^T                      [B,N,M]
    score0  = max_m sim                            [B,N]
    match01 = argmax_m sim                         [B,N]
    match10 = argmax_n sim                         [B,M]
    valid   = (match10[match01[n]] == n) & (score0 > 0.1)
returns (match01, score0, valid).

Key reformulation: the mutual check never needs match10 indices:
    match10[match01[n]] == n  <=>  sim[n, match01[n]] == colmax[match01[n]]
                              <=>  score0[n] == colmax[match01[n]]
(exact fp32 equality is safe because both sides are max-chains over the
same on-device fp32 values; max is exact).

Sharding: 8 cores = 4 batches x 2 row-halves. Each core computes, for its
4096-row slab of one batch:
  - score0 / match01 for its rows (exact, fp32 matmul + max8/max_index)
  - partial column max over its rows [8192]
Host glue: pairwise max of the two partial colmax vectors per batch, then
valid = (score0 > 0.1) & (score0 == colmax[match01]).

Per-core kernel structure (Tile framework):
  for each of 32 n-tiles (128 rows):
    PE:  16 fp32 matmuls  ([128d,128n]^T x [128d,512m] -> PSUM [128,1024] x8)
    ACT: copy each PSUM chunk -> SBUF row buffer [128, 8192]
    DVE: colacc = max(colacc, row)   (column-side accumulate)
    DVE: max8 + max_index on row     (row max + exact first-occurrence argmax)
  GPSIMD: partition_all_reduce(max) over colacc -> partial colmax [8192]
"""

import numpy as np

import concourse.bass as bass  # noqa: F401  (bass must import before tile)
import concourse.mybir as mybir
import concourse.tile as tile
from concourse import bacc, bass_isa

B, N, M, D = 4, 8192, 8192, 128
NCORES = 8
HALF = N // 2          # rows per core
NT = HALF // 128       # 32 n-tiles per core
CW = 1024              # PSUM chunk width (2 banks)
NCHUNK = M // CW       # 8 chunks per n-tile
NEG_INF = -3.0e38

_cached_nc = None


def _build():
    f32 = mybir.dt.float32
    u32 = mybir.dt.uint32
    nc = bacc.Bacc("TRN2", target_bir_lowering=False, debug=False,
                   num_devices=NCORES)
    at = nc.dram_tensor("at", [D, HALF], f32, kind="ExternalInput").ap()
    bt = nc.dram_tensor("bt", [D, M], f32, kind="ExternalInput").ap()
    score_o = nc.dram_tensor("score", [128, NT], f32, kind="ExternalOutput").ap()
    match_o = nc.dram_tensor("match", [128, NT], u32, kind="ExternalOutput").ap()
    colp_o = nc.dram_tensor("colp", [1, M], f32, kind="ExternalOutput").ap()

    with tile.TileContext(nc) as tc:
        with tc.tile_pool(name="big", bufs=1) as big, \
             tc.tile_pool(name="rows", bufs=2) as rows, \
             tc.tile_pool(name="small", bufs=2) as small, \
             tc.tile_pool(name="ps", bufs=3, space="PSUM") as ps:
            atb = big.tile([128, HALF], f32, name="atb")
            btb = big.tile([128, M], f32, name="btb")
            nc.sync.dma_start(atb[:], at[:])
            nc.sync.dma_start(btb[:], bt[:])

            colacc = big.tile([128, M], f32, name="colacc")
            nc.vector.memset(colacc[:], NEG_INF)

            score_all = big.tile([128, NT], f32, name="score_all")
            match_all = big.tile([128, NT], u32, name="match_all")

            for t in range(NT):
                row = rows.tile([128, M], f32, tag="row", name="row")
                for c in range(NCHUNK):
                    pt = ps.tile([128, CW], f32, tag="pt", name="pt")
                    for j in range(2):
                        mlo = c * CW + j * 512
                        nc.tensor.matmul(
                            pt[:, j * 512:(j + 1) * 512],
                            atb[:, t * 128:(t + 1) * 128],
                            btb[:, mlo:mlo + 512],
                            start=True, stop=True)
                    nc.scalar.copy(row[:, c * CW:(c + 1) * CW], pt[:])
                # column-side: running elementwise max over n-tiles
                nc.vector.tensor_tensor(colacc[:], colacc[:], row[:],
                                        op=mybir.AluOpType.max)
                # row-side: top-8 values then first-occurrence index of the max
                mv = small.tile([128, 8], f32, tag="mv", name="mv")
                mi = small.tile([128, 8], u32, tag="mi", name="mi")
                nc.vector.max(mv[:], row[:])
                nc.vector.max_index(mi[:], mv[:], row[:])
                nc.vector.tensor_copy(score_all[:, t:t + 1], mv[:, 0:1])
                nc.vector.tensor_copy(match_all[:, t:t + 1], mi[:, 0:1])

            # partial column max over this core's 4096 rows
            cp = rows.tile([128, M], f32, tag="row", name="cp")
            nc.gpsimd.partition_all_reduce(cp[:], colacc[:], channels=128,
                                           reduce_op=bass_isa.ReduceOp.max)
            nc.sync.dma_start(score_o[:], score_all[:])
            nc.sync.dma_start(match_o[:], match_all[:])
            nc.sync.dma_start(colp_o[:], cp[0:1, :])
    nc.compile()
    return nc


_cached_exec = None


def _build_exec():
    """Compile the NEFF once and return a cached 8-core jitted executable."""
    import jax
    from jax.sharding import Mesh, PartitionSpec
    from jax.experimental.shard_map import shard_map
    from concourse import bass2jax
    from concourse.bass2jax import _bass_exec_p, install_neuronx_cc_hook

    install_neuronx_cc_hook()
    nc = _build()

    partition_name = nc.partition_id_tensor.name if nc.partition_id_tensor else None
    in_names, out_names, out_avals, out_shapes = [], [], [], []
    for alloc in nc.m.functions[0].allocations:
        if not isinstance(alloc, mybir.MemoryLocationSet):
            continue
        name = alloc.memorylocations[0].name
        if alloc.kind == "ExternalInput":
            if name != partition_name:
                in_names.append(name)
        elif alloc.kind == "ExternalOutput":
            shape = tuple(alloc.tensor_shape)
            dtype = mybir.dt.np(alloc.dtype)
            out_names.append(name)
            out_shapes.append((shape, dtype))
            out_avals.append(jax.core.ShapedArray(shape, dtype))
    n_params = len(in_names)
    n_outs = len(out_names)
    all_in_names = in_names + out_names
    if partition_name is not None:
        all_in_names = all_in_names + [partition_name]

    def _body(*args):
        operands = list(args)
        if partition_name is not None:
            operands.append(bass2jax.partition_id_tensor())
        outs = _bass_exec_p.bind(
            *operands, out_avals=tuple(out_avals), in_names=tuple(all_in_names),
            out_names=tuple(out_names), lowering_input_output_aliases=(),
            sim_require_finite=True, sim_require_nnan=True, nc=nc)
        return tuple(outs)

    devices = jax.devices()[:NCORES]
    mesh = Mesh(np.asarray(devices), ("core",))
    in_specs = (PartitionSpec("core"),) * (n_params + n_outs)
    out_specs = (PartitionSpec("core"),) * n_outs
    sharded = jax.jit(
        shard_map(_body, mesh=mesh, in_specs=in_specs, out_specs=out_specs,
                  check_rep=False),
        keep_unused=True)
    return {
        "nc": nc, "fn": sharded, "in_names": in_names,
        "out_names": out_names, "out_shapes": out_shapes,
    }


def kernel(desc0, desc1):
    global _cached_exec
    desc0 = np.asarray(desc0, dtype=np.float32)
    desc1 = np.asarray(desc1, dtype=np.float32)
    assert desc0.shape == (B, N, D) and desc1.shape == (B, M, D)

    if _cached_exec is None:
        _cached_exec = _build_exec()
    ex = _cached_exec

    # build concatenated per-core inputs: axis 0 stacks the 8 cores
    # core = 2*b + h handles rows [h*4096, (h+1)*4096) of batch b
    at_all = np.concatenate(
        [desc0[b, h * HALF:(h + 1) * HALF].T for b in range(B) for h in range(2)],
        axis=0)                                             # [8*128, 4096]
    bt_all = np.concatenate(
        [desc1[b].T for b in range(B) for h in range(2)], axis=0)  # [8*128, 8192]
    ins = {"at": np.ascontiguousarray(at_all), "bt": np.ascontiguousarray(bt_all)}
    concat_in = [ins[n] for n in ex["in_names"]]
    concat_zeros = [np.zeros((NCORES * s[0], *s[1:]), dt)
                    for (s, dt) in ex["out_shapes"]]

    out_arrs = ex["fn"](*concat_in, *concat_zeros)
    res = {}
    for i, name in enumerate(ex["out_names"]):
        shape, dt = ex["out_shapes"][i]
        res[name] = np.asarray(out_arrs[i]).reshape(NCORES, *shape)

    match01 = np.empty((B, N), dtype=np.int32)
    score0 = np.empty((B, N), dtype=np.float32)
    valid = np.empty((B, N), dtype=bool)
    colp = res["colp"].reshape(B, 2, M)
    colmax = colp.max(axis=1)                               # [B, M]

    for core in range(NCORES):
        b, h = divmod(core, 2)
        # score/match stored [partition p, tile t]; row n = t*128 + p
        s = res["score"][core].T.reshape(-1)                # [4096]
        m = res["match"][core].T.reshape(-1).astype(np.int64)
        sl = slice(h * HALF, (h + 1) * HALF)
        score0[b, sl] = s
        match01[b, sl] = m.astype(np.int32)
        valid[b, sl] = (s > 0.1) & (s == colmax[b][m])

    return match01, score0, valid
